# revision 13
# baseline (speedup 1.0000x reference)
"""FBPinn (windowed MoE of per-window tanh MLPs) on 8 Trainium2 cores.

Strategy: data-parallel over the N=65536 collocation points. x is sorted on
the host so every core owns a contiguous x-range; windows whose window
function is below ~1e-6 everywhere in that range are culled per core (the
window fn decays like exp(-d/SIGMA)). All cores run one SPMD program with S
window "slots"; per-core weight tensors select which windows fill the slots
(zero-padded slots contribute exactly 0 via a zero window).

Layout: neurons on SBUF partitions, points on the free axis.

Prologue (per 2048-pt chunk, all hoisted before the slot loops):
  xb     = x broadcast to 128 partitions (ones outer-product on PE ->
           PSUM -> DVE copy to SBUF [128, 2048])
  window = sigmoid((mids_lo-x)/s) * sigmoid((x-mids_hi)/s) computed from a
           64-row broadcast with per-row scale/bias APs on ACT, combined
           on DVE -> [16, 2048] per chunk
Main loop, per chunk and slot:
  h0  = tanh(scale_s * xb + bias_s)    (ACT [128,2048], scale+bias APs)
  h1  = tanh(W1_s.T h0 + b1_s)         (PE matmul -> PSUM, ACT [128,1024])
  h2  = tanh(W2_s.T h1 + b2_s)
  out accumulated into PSUM[16, chunk] row s via a zero-padded M=16 matmul
Tail per chunk: one DVE scalar_tensor_tensor (out + b_out) * window, a
16->1 partition-reduce matmul against a ones vector, DVE copy, DMA out.

Matmul dtypes: the hidden and output layer matmuls run in float32r
(TF32-like 11-bit-mantissa fp32, 4x the fp32 streaming rate); set
HID_F32R / OUT_F32R False for exact-fp32 fallbacks. The input x, the
first-layer affine, all biases, windows, and the final combine stay fp32.
"""

import numpy as np

import concourse.bacc as bacc
import concourse.bass as bass
import concourse.mybir as mybir
import concourse.tile as tile
from concourse import bass_isa
from concourse.bass_utils import run_bass_kernel_spmd

N = 65536
NW = 16
NEUR = 128
SIGMA = 0.02
NCORES = 8
NLOC = N // NCORES  # 8192
CHUNK = 2048
NCHUNK = NLOC // CHUNK  # 4
HALF = 1024
MM = 512  # fp32 moving-operand max free dim

# Window culling: with CUT_SIGMAS=12 the cull error is ~4e-6 relative
# (below fp32 matmul noise). S* becomes 10.
CUT_SIGMAS = 12.0
HID_F32R = True  # hidden-layer matmuls in float32r (TF32-like)
OUT_F32R = True  # output-layer matmul in float32r

F32 = mybir.dt.float32
F32R = mybir.dt.float32r
TANH = mybir.ActivationFunctionType.Tanh
SIG = mybir.ActivationFunctionType.Sigmoid
ADD = mybir.AluOpType.add
MUL = mybir.AluOpType.mult

_cache = {}


def build_nc(S: int):
    """Build the SPMD Bass module with S window slots."""
    HDT = F32R if HID_F32R else F32
    ODT = F32R if OUT_F32R else F32
    nc = bacc.Bacc("TRN2", target_bir_lowering=False, debug=False)

    x_d = nc.dram_tensor("x_loc", [1, NLOC], F32, kind="ExternalInput")
    s0_d = nc.dram_tensor("s0", [NEUR, S], F32, kind="ExternalInput")
    b0_d = nc.dram_tensor("b0", [NEUR, S], F32, kind="ExternalInput")
    w1_d = nc.dram_tensor("w1", [NEUR, S * NEUR], HDT, kind="ExternalInput")
    b1_d = nc.dram_tensor("b1", [NEUR, S], F32, kind="ExternalInput")
    w2_d = nc.dram_tensor("w2", [NEUR, S * NEUR], HDT, kind="ExternalInput")
    b2_d = nc.dram_tensor("b2", [NEUR, S], F32, kind="ExternalInput")
    wo_d = nc.dram_tensor("wo", [NEUR, S * 16], ODT, kind="ExternalInput")
    bo_d = nc.dram_tensor("bo", [16, 1], F32, kind="ExternalInput")
    bsig_d = nc.dram_tensor("bsig", [64, 1], F32, kind="ExternalInput")
    ssig_d = nc.dram_tensor("ssig", [64, 1], F32, kind="ExternalInput")
    y_d = nc.dram_tensor("y", [1, NLOC], F32, kind="ExternalOutput")

    with tile.TileContext(nc) as tc:
        with (
            tc.tile_pool(name="wts", bufs=1) as wp,
            tc.tile_pool(name="xb", bufs=2) as xp,
            tc.tile_pool(name="wn", bufs=2) as vp,
            tc.tile_pool(name="h", bufs=3) as hp,
            tc.tile_pool(name="ps", bufs=2, space="PSUM") as pp,
            tc.tile_pool(name="po", bufs=1, space="PSUM") as op_,
            tc.tile_pool(name="sg", bufs=2) as sp,
            tc.tile_pool(name="tt", bufs=2) as tp,
        ):
            # small consts + x chunk 0 first so prologue work starts ASAP;
            # big weight tensors stream in behind (needed ~20us later).
            x_sb = wp.tile([1, NLOC], F32)
            bsig = wp.tile([64, 1], F32)
            ssig = wp.tile([64, 1], F32)
            s0 = wp.tile([NEUR, S], F32)
            b0 = wp.tile([NEUR, S], F32)
            nc.sync.dma_start(x_sb[0:1, 0:CHUNK], x_d[0:1, 0:CHUNK])
            nc.sync.dma_start(bsig[:], bsig_d[:])
            nc.sync.dma_start(ssig[:], ssig_d[:])
            nc.sync.dma_start(s0[:], s0_d[:])
            nc.sync.dma_start(b0[:], b0_d[:])
            for c in range(1, NCHUNK):
                nc.sync.dma_start(
                    x_sb[0:1, c * CHUNK : (c + 1) * CHUNK],
                    x_d[0:1, c * CHUNK : (c + 1) * CHUNK],
                )
            w1 = wp.tile([NEUR, S * NEUR], HDT)
            nc.sync.dma_start(w1[:], w1_d[:])
            b1 = wp.tile([NEUR, S], F32)
            nc.sync.dma_start(b1[:], b1_d[:])
            w2 = wp.tile([NEUR, S * NEUR], HDT)
            nc.sync.dma_start(w2[:], w2_d[:])
            b2 = wp.tile([NEUR, S], F32)
            nc.sync.dma_start(b2[:], b2_d[:])
            wo = wp.tile([NEUR, S * 16], ODT)
            nc.sync.dma_start(wo[:], wo_d[:])
            bo = wp.tile([16, 1], F32)
            nc.sync.dma_start(bo[:], bo_d[:])

            # ---- prologue builders: x broadcast and window fn per chunk ----
            xbs = {}
            wins = {}

            def emit_prologue(c):
                base = c * CHUNK
                xh = x_sb[0:1, base : base + CHUNK]
                # broadcasts run on the (otherwise idle) GPSIMD engine
                xb = xp.tile([NEUR, CHUNK], F32, tag="xb", name=f"xb{c}")
                nc.gpsimd.partition_broadcast(xb[:], xh, channels=NEUR)
                xbs[c] = xb

                pb = sp.tile([64, CHUNK], F32, tag="sg", name=f"pb{c}")
                nc.gpsimd.partition_broadcast(pb[:], xh, channels=64)
                sg = sp.tile([64, CHUNK], F32, tag="sg", name=f"sg{c}")
                nc.scalar.activation(
                    sg[:], pb[:], SIG, bias=bsig[:, 0:1], scale=ssig[:, 0:1]
                )
                # window = sig_a * sig_b (both direct sigmoids). DVE
                # TensorTensor needs equal SBUF base partitions, so stage
                # sig_b down to partition 0 first.
                win = vp.tile([16, CHUNK], F32, tag="wn", name=f"win{c}")
                sgb = sp.tile([16, CHUNK], F32, tag="sgb", bufs=1, name=f"sgb{c}")
                nc.vector.tensor_copy(sgb[:], sg[32:48, :])
                nc.vector.tensor_mul(win[:], sg[0:16, :], sgb[:])
                wins[c] = win

            for _c in range(NCHUNK):
                emit_prologue(_c)

            # ---- main: per-slot MLPs, outputs accumulated into po rows ----
            def emit_h0(c, s):
                t = hp.tile([NEUR, CHUNK], HDT, tag="h0", bufs=2,
                            name=f"h0_{c}_{s}")
                nc.scalar.activation(
                    t[:], xbs[c][:], TANH,
                    bias=b0[:, s : s + 1], scale=s0[:, s : s + 1],
                )
                return t

            def emit_tail(c, po):
                # y = sum_s window_s * (out_s + b_out_s); 16->1 partition
                # reduce runs on GPSIMD so the PE stream stays pure matmul.
                t2 = tp.tile([16, CHUNK], F32, tag="tt", bufs=1, name=f"t2_{c}")
                nc.vector.scalar_tensor_tensor(
                    t2[:], po[:], bo[:, 0:1], wins[c][:], op0=ADD, op1=MUL
                )
                red = tp.tile([16, CHUNK], F32, tag="rd", name=f"rd{c}")
                nc.gpsimd.partition_all_reduce(
                    red[:], t2[:], 16, bass_isa.ReduceOp.add
                )
                nc.sync.dma_start(
                    y_d[0:1, c * CHUNK : (c + 1) * CHUNK], red[0:1, :]
                )

            h0 = emit_h0(0, 0)
            for c in range(NCHUNK):
                po = op_.tile([16, CHUNK], F32, tag="po", name=f"po{c}")
                for s in range(S):
                    h0_next = None
                    for h in range(2):
                        p1 = pp.tile([NEUR, HALF], F32, tag="ps", name=f"p1_{c}_{s}_{h}")
                        for q in range(2):
                            nc.tensor.matmul(
                                p1[:, q * MM : (q + 1) * MM],
                                w1[:, s * NEUR : (s + 1) * NEUR],
                                h0[:, h * HALF + q * MM : h * HALF + (q + 1) * MM],
                                start=True,
                                stop=True,
                            )
                        h1 = hp.tile([NEUR, HALF], HDT, tag="h1", name=f"h1_{c}_{s}_{h}")
                        nc.scalar.activation(h1[:], p1[:], TANH, bias=b1[:, s : s + 1])
                        p2 = pp.tile([NEUR, HALF], F32, tag="ps", name=f"p2_{c}_{s}_{h}")
                        for q in range(2):
                            nc.tensor.matmul(
                                p2[:, q * MM : (q + 1) * MM],
                                w2[:, s * NEUR : (s + 1) * NEUR],
                                h1[:, q * MM : (q + 1) * MM],
                                start=True,
                                stop=True,
                            )
                        h2 = hp.tile([NEUR, HALF], ODT, tag="h2", name=f"h2_{c}_{s}_{h}")
                        nc.scalar.activation(h2[:], p2[:], TANH, bias=b2[:, s : s + 1])
                        if h == 0:
                            if s + 1 < S:
                                h0_next = emit_h0(c, s + 1)
                            elif c + 1 < NCHUNK:
                                h0_next = emit_h0(c + 1, 0)
                        for q in range(2):
                            nc.tensor.matmul(
                                po[:, h * HALF + q * MM : h * HALF + (q + 1) * MM],
                                wo[:, s * 16 : (s + 1) * 16],
                                h2[:, q * MM : (q + 1) * MM],
                                start=(s == 0),
                                stop=(s == S - 1),
                            )
                    if h0_next is not None:
                        h0 = h0_next
                emit_tail(c, po)

    nc.compile()
    return nc


def _round_f32r(a, enable):
    """Round fp32 to the PE's f32r grid (drop low 12 mantissa bits, RNE)."""
    if not enable:
        return np.ascontiguousarray(a, np.float32)
    b = np.ascontiguousarray(a, np.float32).view(np.uint32).copy()
    lo = b & np.uint32(0xFFF)
    b &= np.uint32(0xFFFFF000)
    rnd = (lo > 0x800) | ((lo == 0x800) & (((b >> np.uint32(12)) & np.uint32(1)) == 1))
    b += rnd.astype(np.uint32) << np.uint32(12)
    return b.view(np.float32)


def _prep_host(x, means, std, mids, W_in, b_in, W_hid, b_hid, W_out, b_out):
    """Sort points, pick per-core windows, build per-core input maps."""
    f32 = np.float32
    xf = np.ascontiguousarray(np.asarray(x, f32).reshape(-1))
    means = np.asarray(means, f32)
    std = np.asarray(std, f32)
    mids = np.asarray(mids, f32)
    W_in = np.asarray(W_in, f32)
    b_in = np.asarray(b_in, f32)
    W_hid = np.asarray(W_hid, f32)
    b_hid = np.asarray(b_hid, f32)
    W_out = np.asarray(W_out, f32)
    b_out = np.asarray(b_out, f32)

    if CUT_SIGMAS is not None:
        order = np.argsort(xf, kind="stable")
    else:
        order = np.arange(N)
    xs = xf[order]
    blocks = xs.reshape(NCORES, NLOC)

    reach = (CUT_SIGMAS * SIGMA) if CUT_SIGMAS is not None else 1e9
    active = []
    for k in range(NCORES):
        lo, hi = blocks[k][0], blocks[k][-1]
        ws = [
            w
            for w in range(NW)
            if (mids[w] - reach) <= hi and (mids[w + 1] + reach) >= lo
        ]
        active.append(ws)
    S = max(len(ws) for ws in active)

    in_maps = []
    for k in range(NCORES):
        ws = active[k]
        s0 = np.zeros((NEUR, S), f32)
        b0 = np.zeros((NEUR, S), f32)
        w1 = np.zeros((NEUR, S * NEUR), f32)
        b1 = np.zeros((NEUR, S), f32)
        w2 = np.zeros((NEUR, S * NEUR), f32)
        b2 = np.zeros((NEUR, S), f32)
        wo = np.zeros((NEUR, S * 16), f32)
        bo = np.zeros((16, 1), f32)
        # pad slots: window identically 0 (both sigmoids 0)
        bsig = np.full((64, 1), -1000.0, f32)
        ssig = np.zeros((64, 1), f32)
        ssig[:16, 0] = -1.0 / SIGMA
        ssig[32:48, 0] = 1.0 / SIGMA
        for s, w in enumerate(ws):
            sc = W_in[w, 0, :] / std[w]
            s0[:, s] = sc
            b0[:, s] = b_in[w] - sc * means[w]
            w1[:, s * NEUR : (s + 1) * NEUR] = W_hid[0, w]
            b1[:, s] = b_hid[0, w]
            w2[:, s * NEUR : (s + 1) * NEUR] = W_hid[1, w]
            b2[:, s] = b_hid[1, w]
            wo[:, s * 16 + s] = W_out[w, :, 0]
            bo[s, 0] = b_out[w, 0]
            # sig_a = sigmoid((mids_lo - x)/SIGMA): scale=-1/s, bias=+mids_lo/s
            bsig[s, 0] = mids[w] / SIGMA
            # sig_b = sigmoid((x - mids_hi)/SIGMA): scale=+1/s, bias=-mids_hi/s
            bsig[32 + s, 0] = -mids[w + 1] / SIGMA
        in_maps.append(
            {
                "x_loc": np.ascontiguousarray(blocks[k][None, :]),
                "s0": s0,
                "b0": b0,
                "w1": _round_f32r(w1, HID_F32R),
                "b1": b1,
                "w2": _round_f32r(w2, HID_F32R),
                "b2": b2,
                "wo": _round_f32r(wo, OUT_F32R),
                "bo": bo,
                "bsig": bsig,
                "ssig": ssig,
            }
        )
    return S, in_maps, order


def get_compiled(S: int):
    if S not in _cache:
        _cache[S] = build_nc(S)
    return _cache[S]


def kernel(**inputs) -> np.ndarray:
    S, in_maps, order = _prep_host(**inputs)
    nc = get_compiled(S)
    res = run_bass_kernel_spmd(nc, in_maps, core_ids=list(range(NCORES)))
    ys = np.concatenate([r["y"].reshape(-1) for r in res.results])
    out = np.empty(N, np.float32)
    out[order] = ys
    return out.reshape(N, 1)


# revision 17
# speedup vs baseline: 1.2326x; 1.2326x over previous
"""FBPinn (windowed MoE of per-window tanh MLPs) on 8 Trainium2 cores.

Strategy: data-parallel over the N=65536 collocation points. x is sorted on
the host so every core owns a contiguous x-range; windows whose window
function is below ~1e-6 everywhere in that range are culled per core (the
window fn decays like exp(-d/SIGMA)). All cores run one SPMD program with S
window "slots"; per-core weight tensors select which windows fill the slots
(zero-padded slots contribute exactly 0 via a zero window).

Layout: neurons on SBUF partitions, points on the free axis.

Prologue (per 2048-pt chunk, all hoisted before the slot loops):
  xb     = x broadcast to 128 partitions (ones outer-product on PE ->
           PSUM -> DVE copy to SBUF [128, 2048])
  window = sigmoid((mids_lo-x)/s) * sigmoid((x-mids_hi)/s) computed from a
           64-row broadcast with per-row scale/bias APs on ACT, combined
           on DVE -> [16, 2048] per chunk
Main loop, per chunk and slot:
  h0  = tanh(scale_s * xb + bias_s)    (ACT [128,2048], scale+bias APs)
  h1  = tanh(W1_s.T h0 + b1_s)         (PE matmul -> PSUM, ACT [128,1024])
  h2  = tanh(W2_s.T h1 + b2_s)
  out accumulated into PSUM[16, chunk] row s via a zero-padded M=16 matmul
Tail per chunk: one DVE scalar_tensor_tensor (out + b_out) * window, a
16->1 partition-reduce matmul against a ones vector, DVE copy, DMA out.

Matmul dtypes: the hidden and output layer matmuls run in float32r
(TF32-like 11-bit-mantissa fp32, 4x the fp32 streaming rate); set
HID_F32R / OUT_F32R False for exact-fp32 fallbacks. The input x, the
first-layer affine, all biases, windows, and the final combine stay fp32.
"""

import numpy as np

import concourse.bacc as bacc
import concourse.bass as bass
import concourse.mybir as mybir
import concourse.tile as tile
from concourse import bass_isa
from concourse.bass_utils import run_bass_kernel_spmd

N = 65536
NW = 16
NEUR = 128
SIGMA = 0.02
NCORES = 8
NLOC = N // NCORES  # 8192
CHUNK = 2048
NCHUNK = NLOC // CHUNK  # 4
HALF = 1024
MM = 512  # fp32 moving-operand max free dim

# Window culling: with CUT_SIGMAS=9 the cull error is ~1.4e-4 relative
# (same order as the f32r matmul error). S* becomes 8.
CUT_SIGMAS = 9.0
HID_F32R = True  # hidden-layer matmuls in float32r (TF32-like)
OUT_F32R = True  # output-layer matmul in float32r

F32 = mybir.dt.float32
F32R = mybir.dt.float32r
TANH = mybir.ActivationFunctionType.Tanh
SIG = mybir.ActivationFunctionType.Sigmoid
ADD = mybir.AluOpType.add
MUL = mybir.AluOpType.mult

_cache = {}


def build_nc(S: int):
    """Build the SPMD Bass module with S window slots."""
    HDT = F32R if HID_F32R else F32
    ODT = F32R if OUT_F32R else F32
    nc = bacc.Bacc("TRN2", target_bir_lowering=False, debug=False)

    x_d = nc.dram_tensor("x_loc", [1, NLOC], F32, kind="ExternalInput")
    s0_d = nc.dram_tensor("s0", [NEUR, S], F32, kind="ExternalInput")
    b0_d = nc.dram_tensor("b0", [NEUR, S], F32, kind="ExternalInput")
    w1_d = nc.dram_tensor("w1", [NEUR, S * NEUR], HDT, kind="ExternalInput")
    b1_d = nc.dram_tensor("b1", [NEUR, S], F32, kind="ExternalInput")
    w2_d = nc.dram_tensor("w2", [NEUR, S * NEUR], HDT, kind="ExternalInput")
    b2_d = nc.dram_tensor("b2", [NEUR, S], F32, kind="ExternalInput")
    wo_d = nc.dram_tensor("wo", [NEUR, S * 16], ODT, kind="ExternalInput")
    bo_d = nc.dram_tensor("bo", [16, 1], F32, kind="ExternalInput")
    bsig_d = nc.dram_tensor("bsig", [64, 1], F32, kind="ExternalInput")
    ssig_d = nc.dram_tensor("ssig", [64, 1], F32, kind="ExternalInput")
    y_d = nc.dram_tensor("y", [1, NLOC], F32, kind="ExternalOutput")

    with tile.TileContext(nc) as tc:
        with (
            tc.tile_pool(name="wts", bufs=1) as wp,
            tc.tile_pool(name="xb", bufs=2) as xp,
            tc.tile_pool(name="wn", bufs=2) as vp,
            tc.tile_pool(name="h", bufs=3) as hp,
            tc.tile_pool(name="ps", bufs=2, space="PSUM") as pp,
            tc.tile_pool(name="po", bufs=1, space="PSUM") as op_,
            tc.tile_pool(name="sg", bufs=2) as sp,
            tc.tile_pool(name="tt", bufs=2) as tp,
        ):
            # small consts + x chunk 0 first so prologue work starts ASAP;
            # big weight tensors stream in behind (needed ~20us later).
            x_sb = wp.tile([1, NLOC], F32)
            bsig = wp.tile([64, 1], F32)
            ssig = wp.tile([64, 1], F32)
            s0 = wp.tile([NEUR, S], F32)
            b0 = wp.tile([NEUR, S], F32)
            nc.sync.dma_start(x_sb[0:1, 0:CHUNK], x_d[0:1, 0:CHUNK])
            nc.sync.dma_start(bsig[:], bsig_d[:])
            nc.sync.dma_start(ssig[:], ssig_d[:])
            nc.sync.dma_start(s0[:], s0_d[:])
            nc.sync.dma_start(b0[:], b0_d[:])
            for c in range(1, NCHUNK):
                nc.sync.dma_start(
                    x_sb[0:1, c * CHUNK : (c + 1) * CHUNK],
                    x_d[0:1, c * CHUNK : (c + 1) * CHUNK],
                )
            w1 = wp.tile([NEUR, S * NEUR], HDT)
            nc.sync.dma_start(w1[:], w1_d[:])
            b1 = wp.tile([NEUR, S], F32)
            nc.sync.dma_start(b1[:], b1_d[:])
            w2 = wp.tile([NEUR, S * NEUR], HDT)
            nc.sync.dma_start(w2[:], w2_d[:])
            b2 = wp.tile([NEUR, S], F32)
            nc.sync.dma_start(b2[:], b2_d[:])
            wo = wp.tile([NEUR, S * 16], ODT)
            nc.sync.dma_start(wo[:], wo_d[:])
            bo = wp.tile([16, 1], F32)
            nc.sync.dma_start(bo[:], bo_d[:])

            # ---- prologue builders: x broadcast and window fn per chunk ----
            xbs = {}
            wins = {}

            def emit_prologue(c):
                base = c * CHUNK
                xh = x_sb[0:1, base : base + CHUNK]
                # broadcasts run on the (otherwise idle) GPSIMD engine
                xb = xp.tile([NEUR, CHUNK], F32, tag="xb", name=f"xb{c}")
                nc.gpsimd.partition_broadcast(xb[:], xh, channels=NEUR)
                xbs[c] = xb

                pb = sp.tile([64, CHUNK], F32, tag="sg", name=f"pb{c}")
                nc.gpsimd.partition_broadcast(pb[:], xh, channels=64)
                sg = sp.tile([64, CHUNK], F32, tag="sg", name=f"sg{c}")
                nc.scalar.activation(
                    sg[:], pb[:], SIG, bias=bsig[:, 0:1], scale=ssig[:, 0:1]
                )
                # window = sig_a * sig_b (both direct sigmoids). DVE
                # TensorTensor needs equal SBUF base partitions, so stage
                # sig_b down to partition 0 first.
                win = vp.tile([16, CHUNK], F32, tag="wn", name=f"win{c}")
                sgb = sp.tile([16, CHUNK], F32, tag="sgb", bufs=1, name=f"sgb{c}")
                nc.vector.tensor_copy(sgb[:], sg[32:48, :])
                nc.vector.tensor_mul(win[:], sg[0:16, :], sgb[:])
                wins[c] = win

            for _c in range(NCHUNK):
                emit_prologue(_c)

            # ---- main: per-slot MLPs, outputs accumulated into po rows ----
            def emit_h0(c, s):
                t = hp.tile([NEUR, CHUNK], HDT, tag="h0", bufs=3,
                            name=f"h0_{c}_{s}")
                nc.scalar.activation(
                    t[:], xbs[c][:], TANH,
                    bias=b0[:, s : s + 1], scale=s0[:, s : s + 1],
                )
                return t

            def emit_tail(c, po):
                # y = sum_s window_s * (out_s + b_out_s); 16->1 partition
                # reduce runs on GPSIMD so the PE stream stays pure matmul.
                t2 = tp.tile([16, CHUNK], F32, tag="tt", bufs=1, name=f"t2_{c}")
                nc.vector.scalar_tensor_tensor(
                    t2[:], po[:], bo[:, 0:1], wins[c][:], op0=ADD, op1=MUL
                )
                red = tp.tile([16, CHUNK], F32, tag="rd", name=f"rd{c}")
                nc.gpsimd.partition_all_reduce(
                    red[:], t2[:], 16, bass_isa.ReduceOp.add
                )
                nc.sync.dma_start(
                    y_d[0:1, c * CHUNK : (c + 1) * CHUNK], red[0:1, :]
                )

            h0 = emit_h0(0, 0)
            for c in range(NCHUNK):
                po = op_.tile([16, CHUNK], F32, tag="po", name=f"po{c}")

                # The out-matmuls read ACT's h2; issuing them immediately puts
                # a PE op that waits on ACT right before the next half's p1
                # matmuls, stalling the ACT->PE->ACT pipeline by ~0.6us per
                # half. Defer each half's out-matmuls until after the NEXT
                # half's p1 block (PSUM accumulation is order-independent
                # within the region, only start/stop placement matters).
                pending_out = []  # (s, h, h2 tile)

                def flush_out(po=po):
                    for ps_, ph_, h2_ in pending_out:
                        for q in range(2):
                            nc.tensor.matmul(
                                po[:, ph_ * HALF + q * MM : ph_ * HALF + (q + 1) * MM],
                                wo[:, ps_ * 16 : (ps_ + 1) * 16],
                                h2_[:, q * MM : (q + 1) * MM],
                                start=(ps_ == 0),
                                stop=(ps_ == S - 1),
                            )
                    pending_out.clear()

                for s in range(S):
                    h0_next = None
                    for h in range(2):
                        p1 = pp.tile([NEUR, HALF], F32, tag="ps", name=f"p1_{c}_{s}_{h}")
                        for q in range(2):
                            nc.tensor.matmul(
                                p1[:, q * MM : (q + 1) * MM],
                                w1[:, s * NEUR : (s + 1) * NEUR],
                                h0[:, h * HALF + q * MM : h * HALF + (q + 1) * MM],
                                start=True,
                                stop=True,
                            )
                        flush_out()
                        h1 = hp.tile([NEUR, HALF], HDT, tag="h1", bufs=4, name=f"h1_{c}_{s}_{h}")
                        nc.scalar.activation(h1[:], p1[:], TANH, bias=b1[:, s : s + 1])
                        p2 = pp.tile([NEUR, HALF], F32, tag="ps", name=f"p2_{c}_{s}_{h}")
                        for q in range(2):
                            nc.tensor.matmul(
                                p2[:, q * MM : (q + 1) * MM],
                                w2[:, s * NEUR : (s + 1) * NEUR],
                                h1[:, q * MM : (q + 1) * MM],
                                start=True,
                                stop=True,
                            )
                        h2 = hp.tile([NEUR, HALF], ODT, tag="h2", bufs=4, name=f"h2_{c}_{s}_{h}")
                        nc.scalar.activation(h2[:], p2[:], TANH, bias=b2[:, s : s + 1])
                        if h == 0:
                            if s + 1 < S:
                                h0_next = emit_h0(c, s + 1)
                            elif c + 1 < NCHUNK:
                                h0_next = emit_h0(c + 1, 0)
                        pending_out.append((s, h, h2))
                        if s == S - 1:
                            # don't defer across the chunk boundary (the po
                            # pool is single-buffered); emit in place
                            flush_out()
                    if h0_next is not None:
                        h0 = h0_next
                emit_tail(c, po)

    nc.compile()
    return nc


def _round_f32r(a, enable):
    """Round fp32 to the PE's f32r grid (drop low 12 mantissa bits, RNE)."""
    if not enable:
        return np.ascontiguousarray(a, np.float32)
    b = np.ascontiguousarray(a, np.float32).view(np.uint32).copy()
    lo = b & np.uint32(0xFFF)
    b &= np.uint32(0xFFFFF000)
    rnd = (lo > 0x800) | ((lo == 0x800) & (((b >> np.uint32(12)) & np.uint32(1)) == 1))
    b += rnd.astype(np.uint32) << np.uint32(12)
    return b.view(np.float32)


def _prep_host(x, means, std, mids, W_in, b_in, W_hid, b_hid, W_out, b_out):
    """Sort points, pick per-core windows, build per-core input maps."""
    f32 = np.float32
    xf = np.ascontiguousarray(np.asarray(x, f32).reshape(-1))
    means = np.asarray(means, f32)
    std = np.asarray(std, f32)
    mids = np.asarray(mids, f32)
    W_in = np.asarray(W_in, f32)
    b_in = np.asarray(b_in, f32)
    W_hid = np.asarray(W_hid, f32)
    b_hid = np.asarray(b_hid, f32)
    W_out = np.asarray(W_out, f32)
    b_out = np.asarray(b_out, f32)

    if CUT_SIGMAS is not None:
        order = np.argsort(xf, kind="stable")
    else:
        order = np.arange(N)
    xs = xf[order]
    blocks = xs.reshape(NCORES, NLOC)

    reach = (CUT_SIGMAS * SIGMA) if CUT_SIGMAS is not None else 1e9
    active = []
    for k in range(NCORES):
        lo, hi = blocks[k][0], blocks[k][-1]
        ws = [
            w
            for w in range(NW)
            if (mids[w] - reach) <= hi and (mids[w + 1] + reach) >= lo
        ]
        active.append(ws)
    S = max(len(ws) for ws in active)

    in_maps = []
    for k in range(NCORES):
        ws = active[k]
        s0 = np.zeros((NEUR, S), f32)
        b0 = np.zeros((NEUR, S), f32)
        w1 = np.zeros((NEUR, S * NEUR), f32)
        b1 = np.zeros((NEUR, S), f32)
        w2 = np.zeros((NEUR, S * NEUR), f32)
        b2 = np.zeros((NEUR, S), f32)
        wo = np.zeros((NEUR, S * 16), f32)
        bo = np.zeros((16, 1), f32)
        # pad slots: window identically 0 (both sigmoids 0)
        bsig = np.full((64, 1), -1000.0, f32)
        ssig = np.zeros((64, 1), f32)
        ssig[:16, 0] = -1.0 / SIGMA
        ssig[32:48, 0] = 1.0 / SIGMA
        for s, w in enumerate(ws):
            sc = W_in[w, 0, :] / std[w]
            s0[:, s] = sc
            b0[:, s] = b_in[w] - sc * means[w]
            w1[:, s * NEUR : (s + 1) * NEUR] = W_hid[0, w]
            b1[:, s] = b_hid[0, w]
            w2[:, s * NEUR : (s + 1) * NEUR] = W_hid[1, w]
            b2[:, s] = b_hid[1, w]
            wo[:, s * 16 + s] = W_out[w, :, 0]
            bo[s, 0] = b_out[w, 0]
            # sig_a = sigmoid((mids_lo - x)/SIGMA): scale=-1/s, bias=+mids_lo/s
            bsig[s, 0] = mids[w] / SIGMA
            # sig_b = sigmoid((x - mids_hi)/SIGMA): scale=+1/s, bias=-mids_hi/s
            bsig[32 + s, 0] = -mids[w + 1] / SIGMA
        in_maps.append(
            {
                "x_loc": np.ascontiguousarray(blocks[k][None, :]),
                "s0": s0,
                "b0": b0,
                "w1": _round_f32r(w1, HID_F32R),
                "b1": b1,
                "w2": _round_f32r(w2, HID_F32R),
                "b2": b2,
                "wo": _round_f32r(wo, OUT_F32R),
                "bo": bo,
                "bsig": bsig,
                "ssig": ssig,
            }
        )
    return S, in_maps, order


def get_compiled(S: int):
    if S not in _cache:
        _cache[S] = build_nc(S)
    return _cache[S]


def kernel(**inputs) -> np.ndarray:
    S, in_maps, order = _prep_host(**inputs)
    nc = get_compiled(S)
    res = run_bass_kernel_spmd(nc, in_maps, core_ids=list(range(NCORES)))
    ys = np.concatenate([r["y"].reshape(-1) for r in res.results])
    out = np.empty(N, np.float32)
    out[order] = ys
    return out.reshape(N, 1)


# revision 18
# speedup vs baseline: 1.4158x; 1.1487x over previous
"""FBPinn (windowed MoE of per-window tanh MLPs) on 8 Trainium2 cores.

Strategy: data-parallel over the N=65536 collocation points. x is sorted on
the host so every core owns a contiguous x-range; windows whose window
function is below ~1e-6 everywhere in that range are culled per core (the
window fn decays like exp(-d/SIGMA)). All cores run one SPMD program with S
window "slots"; per-core weight tensors select which windows fill the slots
(zero-padded slots contribute exactly 0 via a zero window).

Layout: neurons on SBUF partitions, points on the free axis.

Prologue (per 2048-pt chunk, all hoisted before the slot loops):
  xb     = x broadcast to 128 partitions (ones outer-product on PE ->
           PSUM -> DVE copy to SBUF [128, 2048])
  window = sigmoid((mids_lo-x)/s) * sigmoid((x-mids_hi)/s) computed from a
           64-row broadcast with per-row scale/bias APs on ACT, combined
           on DVE -> [16, 2048] per chunk
Main loop, per chunk and slot:
  h0  = tanh(scale_s * xb + bias_s)    (ACT [128,2048], scale+bias APs)
  h1  = tanh(W1_s.T h0 + b1_s)         (PE matmul -> PSUM, ACT [128,1024])
  h2  = tanh(W2_s.T h1 + b2_s)
  out accumulated into PSUM[16, chunk] row s via a zero-padded M=16 matmul
Tail per chunk: one DVE scalar_tensor_tensor (out + b_out) * window, a
16->1 partition-reduce matmul against a ones vector, DVE copy, DMA out.

Matmul dtypes: the hidden and output layer matmuls run in float32r
(TF32-like 11-bit-mantissa fp32, 4x the fp32 streaming rate); set
HID_F32R / OUT_F32R False for exact-fp32 fallbacks. The input x, the
first-layer affine, all biases, windows, and the final combine stay fp32.
"""

import numpy as np

import concourse.bacc as bacc
import concourse.bass as bass
import concourse.mybir as mybir
import concourse.tile as tile
from concourse import bass_isa
from concourse.bass_utils import run_bass_kernel_spmd

N = 65536
NW = 16
NEUR = 128
SIGMA = 0.02
NCORES = 8
NLOC = N // NCORES  # 8192
CHUNK = 2048
NCHUNK = NLOC // CHUNK  # 4
HALF = 1024
MM = 512  # fp32 moving-operand max free dim

# Window culling: with CUT_SIGMAS=9 the cull error is ~1.4e-4 relative
# (same order as the f32r matmul error). S* becomes 8.
CUT_SIGMAS = 9.0
HID_F32R = True  # hidden-layer matmuls in float32r (TF32-like)
OUT_F32R = True  # output-layer matmul in float32r

F32 = mybir.dt.float32
F32R = mybir.dt.float32r
TANH = mybir.ActivationFunctionType.Tanh
SIG = mybir.ActivationFunctionType.Sigmoid
ADD = mybir.AluOpType.add
MUL = mybir.AluOpType.mult

_cache = {}


def build_nc(S: int):
    """Build the SPMD Bass module with S window slots."""
    HDT = F32R if HID_F32R else F32
    ODT = F32R if OUT_F32R else F32
    nc = bacc.Bacc("TRN2", target_bir_lowering=False, debug=False)

    x_d = nc.dram_tensor("x_loc", [1, NLOC], F32, kind="ExternalInput")
    s0_d = nc.dram_tensor("s0", [NEUR, S], F32, kind="ExternalInput")
    b0_d = nc.dram_tensor("b0", [NEUR, S], F32, kind="ExternalInput")
    w1_d = nc.dram_tensor("w1", [NEUR, S * NEUR], HDT, kind="ExternalInput")
    b1_d = nc.dram_tensor("b1", [NEUR, S], F32, kind="ExternalInput")
    w2_d = nc.dram_tensor("w2", [NEUR, S * NEUR], HDT, kind="ExternalInput")
    b2_d = nc.dram_tensor("b2", [NEUR, S], F32, kind="ExternalInput")
    wo_d = nc.dram_tensor("wo", [NEUR, S * 16], ODT, kind="ExternalInput")
    bo_d = nc.dram_tensor("bo", [16, 1], F32, kind="ExternalInput")
    bsig_d = nc.dram_tensor("bsig", [64, 1], F32, kind="ExternalInput")
    ssig_d = nc.dram_tensor("ssig", [64, 1], F32, kind="ExternalInput")
    y_d = nc.dram_tensor("y", [1, NLOC], F32, kind="ExternalOutput")

    with tile.TileContext(nc) as tc:
        with (
            tc.tile_pool(name="wts", bufs=1) as wp,
            tc.tile_pool(name="xb", bufs=2) as xp,
            tc.tile_pool(name="wn", bufs=2) as vp,
            tc.tile_pool(name="h", bufs=3) as hp,
            tc.tile_pool(name="ps", bufs=2, space="PSUM") as pp,
            tc.tile_pool(name="po", bufs=2) as op_,
            tc.tile_pool(name="sg", bufs=2) as sp,
            tc.tile_pool(name="tt", bufs=2) as tp,
        ):
            # small consts + x chunk 0 first so prologue work starts ASAP;
            # big weight tensors stream in behind (needed ~20us later).
            x_sb = wp.tile([1, NLOC], F32)
            bsig = wp.tile([64, 1], F32)
            ssig = wp.tile([64, 1], F32)
            s0 = wp.tile([NEUR, S], F32)
            b0 = wp.tile([NEUR, S], F32)
            nc.sync.dma_start(x_sb[0:1, 0:CHUNK], x_d[0:1, 0:CHUNK])
            nc.sync.dma_start(bsig[:], bsig_d[:])
            nc.sync.dma_start(ssig[:], ssig_d[:])
            nc.sync.dma_start(s0[:], s0_d[:])
            nc.sync.dma_start(b0[:], b0_d[:])
            for c in range(1, NCHUNK):
                nc.sync.dma_start(
                    x_sb[0:1, c * CHUNK : (c + 1) * CHUNK],
                    x_d[0:1, c * CHUNK : (c + 1) * CHUNK],
                )
            w1 = wp.tile([NEUR, S * NEUR], HDT)
            nc.sync.dma_start(w1[:], w1_d[:])
            b1 = wp.tile([NEUR, S], F32)
            nc.sync.dma_start(b1[:], b1_d[:])
            w2 = wp.tile([NEUR, S * NEUR], HDT)
            nc.sync.dma_start(w2[:], w2_d[:])
            b2 = wp.tile([NEUR, S], F32)
            nc.sync.dma_start(b2[:], b2_d[:])
            wo = wp.tile([NEUR, S * 16], ODT)
            nc.sync.dma_start(wo[:], wo_d[:])
            bo = wp.tile([16, 1], F32)
            nc.sync.dma_start(bo[:], bo_d[:])

            # ---- prologue builders: x broadcast and window fn per chunk ----
            xbs = {}
            wins = {}

            def emit_prologue(c):
                base = c * CHUNK
                xh = x_sb[0:1, base : base + CHUNK]
                # broadcasts run on the (otherwise idle) GPSIMD engine
                xb = xp.tile([NEUR, CHUNK], F32, tag="xb", name=f"xb{c}")
                nc.gpsimd.partition_broadcast(xb[:], xh, channels=NEUR)
                xbs[c] = xb

                pb = sp.tile([64, CHUNK], F32, tag="sg", name=f"pb{c}")
                nc.gpsimd.partition_broadcast(pb[:], xh, channels=64)
                sg = sp.tile([64, CHUNK], F32, tag="sg", name=f"sg{c}")
                nc.scalar.activation(
                    sg[:], pb[:], SIG, bias=bsig[:, 0:1], scale=ssig[:, 0:1]
                )
                # window = sig_a * sig_b (both direct sigmoids). DVE
                # TensorTensor needs equal SBUF base partitions, so stage
                # sig_b down to partition 0 first.
                win = vp.tile([16, CHUNK], F32, tag="wn", name=f"win{c}")
                sgb = sp.tile([16, CHUNK], F32, tag="sgb", bufs=1, name=f"sgb{c}")
                nc.vector.tensor_copy(sgb[:], sg[32:48, :])
                nc.vector.tensor_mul(win[:], sg[0:16, :], sgb[:])
                wins[c] = win

            for _c in range(NCHUNK):
                emit_prologue(_c)

            # ---- main: per-slot MLPs, outputs accumulated into po rows ----
            def emit_h0(c, s):
                t = hp.tile([NEUR, CHUNK], HDT, tag="h0", bufs=2,
                            name=f"h0_{c}_{s}")
                nc.scalar.activation(
                    t[:], xbs[c][:], TANH,
                    bias=b0[:, s : s + 1], scale=s0[:, s : s + 1],
                )
                return t

            def emit_tail(c, acc):
                # y = sum_s window_s * (out_s + b_out_s); 16->1 partition
                # reduce runs on GPSIMD so the PE stream stays pure matmul.
                t2 = tp.tile([16, CHUNK], F32, tag="tt", bufs=1, name=f"t2_{c}")
                nc.vector.scalar_tensor_tensor(
                    t2[:], acc[:], bo[:, 0:1], wins[c][:], op0=ADD, op1=MUL
                )
                red = tp.tile([16, CHUNK], F32, tag="rd", name=f"rd{c}")
                nc.gpsimd.partition_all_reduce(
                    red[:], t2[:], 16, bass_isa.ReduceOp.add
                )
                nc.sync.dma_start(
                    y_d[0:1, c * CHUNK : (c + 1) * CHUNK], red[0:1, :]
                )

            h0 = emit_h0(0, 0)
            for c in range(NCHUNK):
                acc = op_.tile([16, CHUNK], F32, tag="po", name=f"acc{c}")
                for s in range(S):
                    p1 = pp.tile([NEUR, CHUNK], F32, tag="ps", name=f"p1_{c}_{s}")
                    for q in range(4):
                        nc.tensor.matmul(
                            p1[:, q * MM : (q + 1) * MM],
                            w1[:, s * NEUR : (s + 1) * NEUR],
                            h0[:, q * MM : (q + 1) * MM],
                            start=True,
                            stop=True,
                        )
                    h1 = hp.tile([NEUR, CHUNK], HDT, tag="h1", bufs=2, name=f"h1_{c}_{s}")
                    nc.scalar.activation(h1[:], p1[:], TANH, bias=b1[:, s : s + 1])
                    p2 = pp.tile([NEUR, CHUNK], F32, tag="ps", name=f"p2_{c}_{s}")
                    for q in range(4):
                        nc.tensor.matmul(
                            p2[:, q * MM : (q + 1) * MM],
                            w2[:, s * NEUR : (s + 1) * NEUR],
                            h1[:, q * MM : (q + 1) * MM],
                            start=True,
                            stop=True,
                        )
                    h2 = hp.tile([NEUR, CHUNK], ODT, tag="h2", bufs=2, name=f"h2_{c}_{s}")
                    nc.scalar.activation(h2[:], p2[:], TANH, bias=b2[:, s : s + 1])
                    if s + 1 < S:
                        h0 = emit_h0(c, s + 1)
                    elif c + 1 < NCHUNK:
                        h0 = emit_h0(c + 1, 0)
                    # out-matmuls reuse p2's PSUM tile (rows 0:16) after ACT
                    # consumed it (WAR dep) - no third PSUM tile in rotation
                    for q in range(4):
                        nc.tensor.matmul(
                            p2[0:16, q * MM : (q + 1) * MM],
                            wo[:, s * 16 : (s + 1) * 16],
                            h2[:, q * MM : (q + 1) * MM],
                            start=True,
                            stop=True,
                        )
                    if s == 0:
                        nc.vector.tensor_copy(acc[:], p2[0:16, :])
                    else:
                        nc.vector.tensor_add(acc[:], acc[:], p2[0:16, :])
                emit_tail(c, acc)

    nc.compile()
    return nc


def _round_f32r(a, enable):
    """Round fp32 to the PE's f32r grid (drop low 12 mantissa bits, RNE)."""
    if not enable:
        return np.ascontiguousarray(a, np.float32)
    b = np.ascontiguousarray(a, np.float32).view(np.uint32).copy()
    lo = b & np.uint32(0xFFF)
    b &= np.uint32(0xFFFFF000)
    rnd = (lo > 0x800) | ((lo == 0x800) & (((b >> np.uint32(12)) & np.uint32(1)) == 1))
    b += rnd.astype(np.uint32) << np.uint32(12)
    return b.view(np.float32)


def _prep_host(x, means, std, mids, W_in, b_in, W_hid, b_hid, W_out, b_out):
    """Sort points, pick per-core windows, build per-core input maps."""
    f32 = np.float32
    xf = np.ascontiguousarray(np.asarray(x, f32).reshape(-1))
    means = np.asarray(means, f32)
    std = np.asarray(std, f32)
    mids = np.asarray(mids, f32)
    W_in = np.asarray(W_in, f32)
    b_in = np.asarray(b_in, f32)
    W_hid = np.asarray(W_hid, f32)
    b_hid = np.asarray(b_hid, f32)
    W_out = np.asarray(W_out, f32)
    b_out = np.asarray(b_out, f32)

    if CUT_SIGMAS is not None:
        order = np.argsort(xf, kind="stable")
    else:
        order = np.arange(N)
    xs = xf[order]
    blocks = xs.reshape(NCORES, NLOC)

    reach = (CUT_SIGMAS * SIGMA) if CUT_SIGMAS is not None else 1e9
    active = []
    for k in range(NCORES):
        lo, hi = blocks[k][0], blocks[k][-1]
        ws = [
            w
            for w in range(NW)
            if (mids[w] - reach) <= hi and (mids[w + 1] + reach) >= lo
        ]
        active.append(ws)
    S = max(len(ws) for ws in active)

    in_maps = []
    for k in range(NCORES):
        ws = active[k]
        s0 = np.zeros((NEUR, S), f32)
        b0 = np.zeros((NEUR, S), f32)
        w1 = np.zeros((NEUR, S * NEUR), f32)
        b1 = np.zeros((NEUR, S), f32)
        w2 = np.zeros((NEUR, S * NEUR), f32)
        b2 = np.zeros((NEUR, S), f32)
        wo = np.zeros((NEUR, S * 16), f32)
        bo = np.zeros((16, 1), f32)
        # pad slots: window identically 0 (both sigmoids 0)
        bsig = np.full((64, 1), -1000.0, f32)
        ssig = np.zeros((64, 1), f32)
        ssig[:16, 0] = -1.0 / SIGMA
        ssig[32:48, 0] = 1.0 / SIGMA
        for s, w in enumerate(ws):
            sc = W_in[w, 0, :] / std[w]
            s0[:, s] = sc
            b0[:, s] = b_in[w] - sc * means[w]
            w1[:, s * NEUR : (s + 1) * NEUR] = W_hid[0, w]
            b1[:, s] = b_hid[0, w]
            w2[:, s * NEUR : (s + 1) * NEUR] = W_hid[1, w]
            b2[:, s] = b_hid[1, w]
            wo[:, s * 16 + s] = W_out[w, :, 0]
            bo[s, 0] = b_out[w, 0]
            # sig_a = sigmoid((mids_lo - x)/SIGMA): scale=-1/s, bias=+mids_lo/s
            bsig[s, 0] = mids[w] / SIGMA
            # sig_b = sigmoid((x - mids_hi)/SIGMA): scale=+1/s, bias=-mids_hi/s
            bsig[32 + s, 0] = -mids[w + 1] / SIGMA
        in_maps.append(
            {
                "x_loc": np.ascontiguousarray(blocks[k][None, :]),
                "s0": s0,
                "b0": b0,
                "w1": _round_f32r(w1, HID_F32R),
                "b1": b1,
                "w2": _round_f32r(w2, HID_F32R),
                "b2": b2,
                "wo": _round_f32r(wo, OUT_F32R),
                "bo": bo,
                "bsig": bsig,
                "ssig": ssig,
            }
        )
    return S, in_maps, order


def get_compiled(S: int):
    if S not in _cache:
        _cache[S] = build_nc(S)
    return _cache[S]


def kernel(**inputs) -> np.ndarray:
    S, in_maps, order = _prep_host(**inputs)
    nc = get_compiled(S)
    res = run_bass_kernel_spmd(nc, in_maps, core_ids=list(range(NCORES)))
    ys = np.concatenate([r["y"].reshape(-1) for r in res.results])
    out = np.empty(N, np.float32)
    out[order] = ys
    return out.reshape(N, 1)
